# revision 35
# baseline (speedup 1.0000x reference)
"""Trainium2 Bass kernel: Conv2d(1->64, k=7, valid) on data [32,1,224,224] f32.

Data-parallel over batch (4 images per core on 8 cores).  Per core:
im2col matmul in fp16 (fp32 PSUM), output stored as fp16 and cast back
on the host (lossless for this value range).

Design (from trace analysis; ~1.27x over the previous kernel):
  - Matmul cost on TRN2 is column-streaming-bound: a [K,128]x[K,448]
    matmul costs the same for K = 28..98.  So each PSUM chunk computes
    TWO tiles at once: the pair's im2col rows are stacked vertically
    (kp = 2*7*KXL partitions) and the weights are block-diagonal
    [kp, 128], yielding both tiles' 64 channels in one [128,448] bank.
  - KXL (materialized kx shifts) trades im2col DMA bytes (~2.8*KXL
    MB/core) against matmuls per chunk (ceil(7/KXL)).  Main rounds use
    KXL=2 (5.6 MB in); the 4x matmul count is hidden by running 4
    pair-streams CONCURRENTLY on the 4 PE row-strips via explicit
    tile_position=(32p, 0) (kp=28 rows fits one 32-row strip).  Matmul
    issue is interleaved across pairs (p inner, m outer) so concurrent
    streams never head-of-line block on a busy strip.  Concurrency
    requires per-pair PSUM tiles: two pairs sharing one PSUM tile
    serializes their matmuls in the tile scheduler (measured 160us).
  - The kernel is co-limited by the HBM store stream (~24.3 MB fp16
    out at ~360-390 GB/s) and the chunk cadence (16 matmuls 4-way +
    2x ~550ns PSUM->SBUF copies on DVE+ACT per chunk-round, ~1.1us).
    The first rounds are small with high KXL ((7,1) then (4,2)): first
    stores issue ~21us in without paying the serial-matmul cost that
    full-size KXL=2 rounds would have cold (the PE HAM clock gate runs
    1.2 GHz until ~3.4us of sustained activity).  The tail rounds
    shrink ((2,3),(2,2)) so the final store drain is ~3 MB.
  - Stores are full-tile [64ch, 28x218] (12.2 KB/channel descriptors
    sustain ~390 GB/s; half-tile stores measured ~30% slower).  Loads
    go on gpsimd (SWDGE), stores on sync (HWDGE qSP): mixing stores
    onto gpsimd or scalar queues measured slower.
"""

import numpy as np

B = 32            # full batch
OC = 64           # out channels
KS = 7            # kernel size
H = 224           # input H=W
OH = 218          # valid output rows/cols
OW = 224          # im2col row width (incl 6 garbage cols)
NCORES = 8
IPC = B // NCORES  # images per core

BLK = 28          # output rows per tile
NBLK = 8          # tiles per image (7x28 + 1x22 valid rows)
NTILES = IPC * NBLK
NPAIRS = NTILES // 2
NCOLS = BLK * OW  # 6272 im2col columns per tile
RUN = NCOLS + 8   # per-partition run (covers kx shifts)
CHUNK = 448       # psum chunk columns (= 2 output rows)
NCHUNK = NCOLS // CHUNK  # 14
OBW = CHUNK // OW * OH   # 436 ob columns per chunk (garbage stripped)

# rounds: (kxl, npairs).  Each round processes npairs pairs as
# concurrent PE row-strip streams (kxl=2: up to 4, kxl=4: up to 2,
# kxl=7: 1).  Early rounds are small so the first output stores issue
# as soon as possible (the kernel is bound by the HBM store stream,
# which must start early to stay saturated) and use high kxl so their
# serial matmul count stays low (kxl=7 pair: 14 matmuls vs 56 at
# kxl=2).  Later rounds use kxl=2 (4x less im2col DMA than kxl=7,
# matmul count hidden by 4-way row-strip concurrency).
# sum of npairs must equal NPAIRS.
ROUNDS = [(7, 1), (4, 2), (2, 4), (2, 4), (2, 3), (2, 2)]

# weight DRAM layout: one [128, 896] tensor; col offset per kxl variant
WOFF = {2: 0, 4: 512, 7: 768}

_CACHE = {}


def _pairs_per_round(kxl):
    return {2: 4, 4: 2, 7: 1}[kxl]


def _nmat(kxl):
    return -(-KS // kxl)


def _build():
    import concourse.mybir as mybir
    import concourse.tile as tile
    from concourse import bacc
    import concourse.bass as bass

    assert sum(n for _, n in ROUNDS) == NPAIRS

    nc = bacc.Bacc("TRN2", target_bir_lowering=False, debug=False)

    total_rows = sum(14 * k * n for k, n in ROUNDS)
    i2cd = nc.dram_tensor("i2cd", [total_rows, RUN], mybir.dt.float16,
                          kind="ExternalInput")
    wbd = nc.dram_tensor("wbd", [128, 960], mybir.dt.float16,
                         kind="ExternalInput")
    out = nc.dram_tensor("out", [IPC, OC, OH, OH], mybir.dt.float16,
                         kind="ExternalOutput")

    with tile.TileContext(nc) as tc:
        with (
            tc.tile_pool(name="wp", bufs=1) as w_pool,
            tc.tile_pool(name="i2c", bufs=3) as i2c_pool,
            tc.tile_pool(name="ob", bufs=8) as ob_pool,
            tc.tile_pool(name="ps", bufs=8, space="PSUM") as ps_pool,
        ):
            wt = w_pool.tile([128, 960], mybir.dt.float16)
            nc.sync.dma_start(out=wt[:, :], in_=wbd[:, :])

            # SBUF strip bases per round size
            SB = {1: [0], 2: [0, 64], 3: [0, 32, 64], 4: [0, 32, 64, 96]}

            # precompute per-round pair row offsets in i2cd
            rounds = []
            rowbase = 0
            pair0 = 0
            for kxl, np_r in ROUNDS:
                kp = 14 * kxl
                assert kp <= 128 // np_r
                bases = []
                for p in range(np_r):
                    bases.append(rowbase)
                    rowbase += kp
                rounds.append((kxl, np_r, kp, pair0, bases))
                pair0 += np_r

            i2c_tiles = {}

            def issue_in(r):
                kxl, np_r, kp, _, bases = rounds[r]
                i2c = i2c_pool.tile([128, RUN], mybir.dt.float16,
                                    tag="i2c", name=f"i2c{r}")
                if kxl == KS:
                    # split pair: tile A at strips 0-1, tile B at 2-3
                    nc.gpsimd.dma_start(
                        out=i2c[0:49, :], in_=i2cd[bases[0]: bases[0] + 49, :])
                    nc.gpsimd.dma_start(
                        out=i2c[64:113, :],
                        in_=i2cd[bases[0] + 49: bases[0] + 98, :])
                else:
                    for p in range(np_r):
                        sb = SB[np_r][p]
                        nc.gpsimd.dma_start(
                            out=i2c[sb: sb + kp, :],
                            in_=i2cd[bases[p]: bases[p] + kp, :])
                i2c_tiles[r] = i2c

            PF = 3
            for r in range(min(PF, len(rounds))):
                issue_in(r)

            copy_k = 0
            for r in range(len(rounds)):
                kxl, np_r, kp, pair0, _ = rounds[r]
                nmat = _nmat(kxl)
                woff = WOFF[kxl]

                if r + PF < len(rounds):
                    issue_in(r + PF)
                i2c = i2c_tiles.pop(r)

                obs = [ob_pool.tile([128, NCHUNK * OBW], mybir.dt.float16,
                                    tag="ob", name=f"ob{r}_{p}")
                       for p in range(np_r)]

                for j in range(NCHUNK):
                    c0 = CHUNK * j
                    if kxl == KS:
                        # two concurrent single-tile streams (M=64 each):
                        # A on row strips 0-1 / psum cols 0-1, B on row
                        # strips 2-3 / psum cols 2-3 -> disjoint PE cells
                        psa = ps_pool.tile([128, CHUNK], mybir.dt.float32,
                                           tag="ps", name=f"psa{r}_{j}")
                        psb = ps_pool.tile([128, CHUNK], mybir.dt.float32,
                                           tag="ps", name=f"psb{r}_{j}")
                        nc.tensor.matmul(
                            psa[0:64, :], wt[0:49, 768:832],
                            i2c[0:49, c0: c0 + CHUNK],
                            start=True, stop=True, tile_position=(0, 0))
                        nc.tensor.matmul(
                            psb[64:128, :], wt[64:113, 896:960],
                            i2c[64:113, c0: c0 + CHUNK],
                            start=True, stop=True, tile_position=(64, 64))
                        for half, ps in ((0, psa), (1, psb)):
                            sl = ps[64 * half: 64 * half + 64, :]
                            pssrc = bass.AP(
                                tensor=sl.tensor, offset=sl.offset,
                                ap=[[sl.ap[0][0], 64], [OW, CHUNK // OW],
                                    [1, OH]],
                            )
                            dst = obs[0][64 * half: 64 * half + 64,
                                         OBW * j: OBW * (j + 1)]
                            if half == 0:
                                nc.vector.tensor_copy(dst, pssrc)
                            else:
                                nc.scalar.copy(dst, pssrc)
                            copy_k += 1
                        continue
                    pss = [ps_pool.tile([128, CHUNK], mybir.dt.float32,
                                        tag="ps", name=f"ps{r}_{j}_{p}")
                           for p in range(np_r)]
                    for m in range(nmat):
                        st, sp = (m == 0), (m == nmat - 1)
                        for p in range(np_r):
                            sb = SB[np_r][p]
                            off = c0 + kxl * m
                            nc.tensor.matmul(
                                pss[p][:, :],
                                wt[sb: sb + kp,
                                   woff + 128 * m: woff + 128 * (m + 1)],
                                i2c[sb: sb + kp, off: off + CHUNK],
                                start=st, stop=sp,
                                tile_position=(sb, 0),
                            )
                    # strip the 6 garbage cols of each 224-col output row
                    for p in range(np_r):
                        ps = pss[p]
                        pssrc = bass.AP(
                            tensor=ps.tensor, offset=ps.offset,
                            ap=[[ps.ap[0][0], 128], [OW, CHUNK // OW],
                                [1, OH]],
                        )
                        dst = obs[p][:, OBW * j: OBW * (j + 1)]
                        if copy_k % 2 == 0:
                            nc.vector.tensor_copy(dst, pssrc)
                        else:
                            nc.scalar.copy(dst, pssrc)
                        copy_k += 1

                for p in range(np_r):
                    for half in range(2):
                        t = 2 * (pair0 + p) + half
                        imgi, blk = divmod(t, NBLK)
                        r0 = BLK * blk
                        nrows = min(BLK, OH - r0)
                        nc.sync.dma_start(
                            out=out[imgi, :, r0: r0 + nrows, :],
                            in_=obs[p][64 * half: 64 * half + OC,
                                       : nrows * OH])

    nc.compile()
    return nc


def _prep_inputs(data, weight):
    d = np.asarray(data).reshape(B, H, H).astype(np.float16)
    dpad = np.zeros((B, 256, H), dtype=np.float16)
    dpad[:, :H, :] = d
    dflat = dpad.reshape(B, 256 * H)
    w = np.asarray(weight).reshape(OC, KS, KS).astype(np.float16)

    # weights: block-diagonal stacked-pair layout per kxl variant.
    # variant kxl, matmul m: W_m[kp_t*h + (kxl*ky + kxi), 64*h + oc]
    #   = w[oc, ky, kxi + kxl*m]  (0 when kx >= 7)
    wbd = np.zeros((128, 960), dtype=np.float16)
    # single-tile kxl7 weights for the split first round's B stream
    wbd[64:113, 896:960] = w.reshape(OC, KS * KS).T
    for kxl in (2, 4, 7):
        kp_t = KS * kxl
        nmat = _nmat(kxl)
        np_r = _pairs_per_round(kxl)
        sbase = 128 // np_r
        blk_w = np.zeros((nmat, 2 * kp_t, 128), dtype=np.float16)
        for m in range(nmat):
            for ky in range(KS):
                for kxi in range(kxl):
                    kx = kxi + kxl * m
                    if kx >= KS:
                        continue
                    u = kxl * ky + kxi
                    for h in range(2):
                        blk_w[m, kp_t * h + u, 64 * h: 64 * h + 64] = \
                            w[:, ky, kx]
        for s in range(np_r):
            for m in range(nmat):
                wbd[sbase * s: sbase * s + 2 * kp_t,
                    WOFF[kxl] + 128 * m: WOFF[kxl] + 128 * (m + 1)] = \
                    blk_w[m]

    # im2col rows, KXL-materialized, pairs stacked (A rows then B rows)
    total_rows = sum(14 * k * n for k, n in ROUNDS)
    pair_kxl = []
    for kxl, n in ROUNDS:
        pair_kxl += [kxl] * n

    in_maps = []
    for c in range(NCORES):
        i2cd = np.empty((total_rows, RUN), dtype=np.float16)
        sw = np.lib.stride_tricks.sliding_window_view(dflat, RUN, axis=1)
        rb = 0
        for P in range(NPAIRS):
            kxl = pair_kxl[P]
            for half in range(2):
                t = 2 * P + half
                imgi, blk = divmod(t, NBLK)
                g = c * IPC + imgi
                r0 = BLK * blk
                starts = np.array(
                    [(r0 + ky) * H + kxi
                     for ky in range(KS) for kxi in range(kxl)])
                i2cd[rb: rb + KS * kxl] = sw[g][starts]
                rb += KS * kxl
        assert rb == total_rows
        in_maps.append({"i2cd": i2cd, "wbd": wbd})
    return in_maps


def kernel(data, weight):
    from concourse.bass_utils import run_bass_kernel_spmd

    if "nc" not in _CACHE:
        _CACHE["nc"] = _build()
    nc = _CACHE["nc"]

    in_maps = _prep_inputs(np.asarray(data), np.asarray(weight))
    res = run_bass_kernel_spmd(nc, in_maps, core_ids=list(range(NCORES)))
    outs = [r["out"] for r in res.results]
    full = np.concatenate(outs, axis=0)  # [32, 64, 218, 218] f16
    return full.astype(np.float32)


# revision 36
# speedup vs baseline: 1.1401x; 1.1401x over previous
"""Trainium2 Bass kernel: Conv2d(1->64, k=7, valid) on data [32,1,224,224] f32.

Data-parallel over batch (4 images per core on 8 cores).  Per core:
im2col matmul in fp16 (fp32 PSUM), output stored as fp16 and cast back
on the host (lossless for this value range).

Design (from trace analysis; ~1.27x over the previous kernel):
  - Matmul cost on TRN2 is column-streaming-bound: a [K,128]x[K,448]
    matmul costs the same for K = 28..98.  So each PSUM chunk computes
    TWO tiles at once: the pair's im2col rows are stacked vertically
    (kp = 2*7*KXL partitions) and the weights are block-diagonal
    [kp, 128], yielding both tiles' 64 channels in one [128,448] bank.
  - KXL (materialized kx shifts) trades im2col DMA bytes (~2.8*KXL
    MB/core) against matmuls per chunk (ceil(7/KXL)).  Main rounds use
    KXL=2 (5.6 MB in); the 4x matmul count is hidden by running 4
    pair-streams CONCURRENTLY on the 4 PE row-strips via explicit
    tile_position=(32p, 0) (kp=28 rows fits one 32-row strip).  Matmul
    issue is interleaved across pairs (p inner, m outer) so concurrent
    streams never head-of-line block on a busy strip.  Concurrency
    requires per-pair PSUM tiles: two pairs sharing one PSUM tile
    serializes their matmuls in the tile scheduler (measured 160us).
  - The kernel is co-limited by the HBM store stream (~24.3 MB fp16
    out at ~360-390 GB/s) and the chunk cadence (16 matmuls 4-way +
    2x ~550ns PSUM->SBUF copies on DVE+ACT per chunk-round, ~1.1us).
    The first rounds are small with high KXL ((7,1) then (4,2)): first
    stores issue ~21us in without paying the serial-matmul cost that
    full-size KXL=2 rounds would have cold (the PE HAM clock gate runs
    1.2 GHz until ~3.4us of sustained activity).  The tail rounds
    shrink ((2,3),(2,2)) so the final store drain is ~3 MB.
  - Stores are full-tile [64ch, 28x218] (12.2 KB/channel descriptors
    sustain ~390 GB/s; half-tile stores measured ~30% slower).  Loads
    go on gpsimd (SWDGE), stores on sync (HWDGE qSP): mixing stores
    onto gpsimd or scalar queues measured slower.
"""

import numpy as np

B = 32            # full batch
OC = 64           # out channels
KS = 7            # kernel size
H = 224           # input H=W
OH = 218          # valid output rows/cols
OW = 224          # im2col row width (incl 6 garbage cols)
NCORES = 8
IPC = B // NCORES  # images per core

BLK = 28          # output rows per tile
NBLK = 8          # tiles per image (7x28 + 1x22 valid rows)
NTILES = IPC * NBLK
NPAIRS = NTILES // 2
NCOLS = BLK * OW  # 6272 im2col columns per tile
RUN = NCOLS + 8   # per-partition run (covers kx shifts)
CHUNK = 448       # psum chunk columns (= 2 output rows)
NCHUNK = NCOLS // CHUNK  # 14
OBW = CHUNK // OW * OH   # 436 ob columns per chunk (garbage stripped)

# rounds: (kxl, npairs).  Each round processes npairs pairs as
# concurrent PE row-strip streams (kxl=2: up to 4, kxl=4: up to 2,
# kxl=7: 1).  Early rounds are small so the first output stores issue
# as soon as possible (the kernel is bound by the HBM store stream,
# which must start early to stay saturated) and use high kxl so their
# serial matmul count stays low (kxl=7 pair: 14 matmuls vs 56 at
# kxl=2).  Later rounds use kxl=2 (4x less im2col DMA than kxl=7,
# matmul count hidden by 4-way row-strip concurrency).
# sum of npairs must equal NPAIRS.
ROUNDS = [(7, 1), (4, 2), (2, 4), (2, 4), (2, 3), (2, 2)]

# weight DRAM layout: one [128, 896] tensor; col offset per kxl variant
WOFF = {2: 0, 4: 512, 7: 768}

_CACHE = {}


def _pairs_per_round(kxl):
    return {2: 4, 4: 2, 7: 1}[kxl]


def _nmat(kxl):
    return -(-KS // kxl)


def _build():
    import concourse.mybir as mybir
    import concourse.tile as tile
    from concourse import bacc
    import concourse.bass as bass

    assert sum(n for _, n in ROUNDS) == NPAIRS

    nc = bacc.Bacc("TRN2", target_bir_lowering=False, debug=False)

    total_rows = sum(14 * k * n for k, n in ROUNDS)
    i2cd = nc.dram_tensor("i2cd", [total_rows, RUN], mybir.dt.float16,
                          kind="ExternalInput")
    wbd = nc.dram_tensor("wbd", [128, 896], mybir.dt.float16,
                         kind="ExternalInput")
    out = nc.dram_tensor("out", [IPC, OC, OH, OH], mybir.dt.float16,
                         kind="ExternalOutput")

    with tile.TileContext(nc) as tc:
        with (
            tc.tile_pool(name="wp", bufs=1) as w_pool,
            tc.tile_pool(name="i2c", bufs=3) as i2c_pool,
            tc.tile_pool(name="ob", bufs=8) as ob_pool,
            tc.tile_pool(name="ps", bufs=8, space="PSUM") as ps_pool,
        ):
            wt = w_pool.tile([128, 896], mybir.dt.float16)
            nc.sync.dma_start(out=wt[:, :], in_=wbd[:, :])

            # SBUF strip bases per round size
            SB = {1: [0], 2: [0, 64], 3: [0, 32, 64], 4: [0, 32, 64, 96]}

            # precompute per-round pair row offsets in i2cd
            rounds = []
            rowbase = 0
            pair0 = 0
            for kxl, np_r in ROUNDS:
                kp = 14 * kxl
                assert kp <= 128 // np_r
                bases = []
                for p in range(np_r):
                    bases.append(rowbase)
                    rowbase += kp
                rounds.append((kxl, np_r, kp, pair0, bases))
                pair0 += np_r

            i2c_tiles = {}

            def issue_in(r):
                kxl, np_r, kp, _, bases = rounds[r]
                i2c = i2c_pool.tile([128, RUN], mybir.dt.float16,
                                    tag="i2c", name=f"i2c{r}")
                for p in range(np_r):
                    sb = SB[np_r][p]
                    nc.gpsimd.dma_start(
                        out=i2c[sb: sb + kp, :],
                        in_=i2cd[bases[p]: bases[p] + kp, :])
                i2c_tiles[r] = i2c

            PF = 3
            for r in range(min(PF, len(rounds))):
                issue_in(r)

            copy_k = 0
            for r in range(len(rounds)):
                kxl, np_r, kp, pair0, _ = rounds[r]
                nmat = _nmat(kxl)
                woff = WOFF[kxl]

                if r + PF < len(rounds):
                    issue_in(r + PF)
                i2c = i2c_tiles.pop(r)

                obs = [ob_pool.tile([128, NCHUNK * OBW], mybir.dt.float16,
                                    tag="ob", name=f"ob{r}_{p}")
                       for p in range(np_r)]

                for j in range(NCHUNK):
                    c0 = CHUNK * j
                    pss = [ps_pool.tile([128, CHUNK], mybir.dt.float32,
                                        tag="ps", name=f"ps{r}_{j}_{p}")
                           for p in range(np_r)]
                    for m in range(nmat):
                        st, sp = (m == 0), (m == nmat - 1)
                        for p in range(np_r):
                            sb = SB[np_r][p]
                            off = c0 + kxl * m
                            nc.tensor.matmul(
                                pss[p][:, :],
                                wt[sb: sb + kp,
                                   woff + 128 * m: woff + 128 * (m + 1)],
                                i2c[sb: sb + kp, off: off + CHUNK],
                                start=st, stop=sp,
                                tile_position=(sb, 0),
                            )
                    # strip the 6 garbage cols of each 224-col output row
                    for p in range(np_r):
                        ps = pss[p]
                        pssrc = bass.AP(
                            tensor=ps.tensor, offset=ps.offset,
                            ap=[[ps.ap[0][0], 128], [OW, CHUNK // OW],
                                [1, OH]],
                        )
                        dst = obs[p][:, OBW * j: OBW * (j + 1)]
                        if copy_k % 2 == 0:
                            nc.vector.tensor_copy(dst, pssrc)
                        else:
                            nc.scalar.copy(dst, pssrc)
                        copy_k += 1

                for p in range(np_r):
                    for half in range(2):
                        t = 2 * (pair0 + p) + half
                        imgi, blk = divmod(t, NBLK)
                        r0 = BLK * blk
                        nrows = min(BLK, OH - r0)
                        nc.sync.dma_start(
                            out=out[imgi, :, r0: r0 + nrows, :],
                            in_=obs[p][64 * half: 64 * half + OC,
                                       : nrows * OH])

    nc.compile()
    return nc


def _prep_inputs(data, weight):
    d = np.asarray(data).reshape(B, H, H).astype(np.float16)
    dpad = np.zeros((B, 256, H), dtype=np.float16)
    dpad[:, :H, :] = d
    dflat = dpad.reshape(B, 256 * H)
    w = np.asarray(weight).reshape(OC, KS, KS).astype(np.float16)

    # weights: block-diagonal stacked-pair layout per kxl variant.
    # variant kxl, matmul m: W_m[kp_t*h + (kxl*ky + kxi), 64*h + oc]
    #   = w[oc, ky, kxi + kxl*m]  (0 when kx >= 7)
    wbd = np.zeros((128, 896), dtype=np.float16)
    for kxl in (2, 4, 7):
        kp_t = KS * kxl
        nmat = _nmat(kxl)
        np_r = _pairs_per_round(kxl)
        sbase = 128 // np_r
        blk_w = np.zeros((nmat, 2 * kp_t, 128), dtype=np.float16)
        for m in range(nmat):
            for ky in range(KS):
                for kxi in range(kxl):
                    kx = kxi + kxl * m
                    if kx >= KS:
                        continue
                    u = kxl * ky + kxi
                    for h in range(2):
                        blk_w[m, kp_t * h + u, 64 * h: 64 * h + 64] = \
                            w[:, ky, kx]
        for s in range(np_r):
            for m in range(nmat):
                wbd[sbase * s: sbase * s + 2 * kp_t,
                    WOFF[kxl] + 128 * m: WOFF[kxl] + 128 * (m + 1)] = \
                    blk_w[m]

    # im2col rows, KXL-materialized, pairs stacked (A rows then B rows)
    total_rows = sum(14 * k * n for k, n in ROUNDS)
    pair_kxl = []
    for kxl, n in ROUNDS:
        pair_kxl += [kxl] * n

    in_maps = []
    for c in range(NCORES):
        i2cd = np.empty((total_rows, RUN), dtype=np.float16)
        sw = np.lib.stride_tricks.sliding_window_view(dflat, RUN, axis=1)
        rb = 0
        for P in range(NPAIRS):
            kxl = pair_kxl[P]
            for half in range(2):
                t = 2 * P + half
                imgi, blk = divmod(t, NBLK)
                g = c * IPC + imgi
                r0 = BLK * blk
                starts = np.array(
                    [(r0 + ky) * H + kxi
                     for ky in range(KS) for kxi in range(kxl)])
                i2cd[rb: rb + KS * kxl] = sw[g][starts]
                rb += KS * kxl
        assert rb == total_rows
        in_maps.append({"i2cd": i2cd, "wbd": wbd})
    return in_maps


def kernel(data, weight):
    from concourse.bass_utils import run_bass_kernel_spmd

    if "nc" not in _CACHE:
        _CACHE["nc"] = _build()
    nc = _CACHE["nc"]

    in_maps = _prep_inputs(np.asarray(data), np.asarray(weight))
    res = run_bass_kernel_spmd(nc, in_maps, core_ids=list(range(NCORES)))
    outs = [r["out"] for r in res.results]
    full = np.concatenate(outs, axis=0)  # [32, 64, 218, 218] f16
    return full.astype(np.float32)
